# revision 1
# baseline (speedup 1.0000x reference)
"""Multi-head attention (softmax over the QUERY axis) on 8 TRN2 NeuronCores.

Sharding: 2 batches x 4 head-groups (4 heads each) -> 8 cores.
Each core computes, for its (batch b, heads 4g..4g+3):
    qkT = W_{q,k} @ x_b^T + b_{q,k}   [512, 2048]   (e_out on partitions)
    V   = x_b @ W_v^T + b_v           [2048, 256]
    S'  = K Q^T (scores TRANSPOSED)   [k, q] per head
    P   = exp(S'/8) with fused row-sum -> denom[k]  (softmax over q == free dim)
    outT= sum_k (V[k,:]/denom[k]) P[k,:]            [d, q] per head
    part= outT^T @ WoT_g              [2048, 1024]  (partial for this head group)
Host sums the 4 partials per batch and adds bo (the tensor-parallel epilogue).

Matmul inputs are bf16 (PSUM accumulation, softmax statistics and V'/denom
math stay fp32); host pre-casts x/W to bf16 (halves input DMA). Head pairs
share the PE array via disjoint row groups (scores: d at partitions 0/64)
and disjoint column groups (attn.V: outT partitions 0/64).

Pipelining: only the Q/K tiles for head-pair 0 are computed up front; the
remaining qkT/V work is emitted as PE "filler" groups interleaved into
pair 0's attention k-loop, so the PE never idles while ACT paces the
exp stream. attn.V accumulates in a 2-bank transient PSUM tile per
(4-ktile group, q-half) and flushes into an SBUF fp32 accumulator, keeping
total PSUM at 8 banks: S'(2x2) + attn.V(2) + qkv/final groups(2).
"""

import sys

if "/opt/trn_rl_repo" not in sys.path:
    sys.path.insert(0, "/opt/trn_rl_repo")

import numpy as np
import ml_dtypes

import concourse.bass as bass
import concourse.mybir as mybir
import concourse.tile as tile
from concourse import bacc
from concourse.bass_utils import run_bass_kernel_spmd

F32 = mybir.dt.float32
BF16 = mybir.dt.bfloat16
AF = mybir.ActivationFunctionType

B, S, E, H = 2, 2048, 1024, 16
HL = 4  # heads per core
DH = 64
QK = 512  # q+k out dims per core (2*HL*DH)
V3 = 768  # q+k+v out dims per core
NCORES = 8

ET = E // 128  # 8 e-tiles
ST = S // 128  # 16 s-tiles
SC = S // 512  # 4 s/q chunks of 512
KT = ST  # 16 k-tiles
FG = 4  # k-tiles per attn.V accumulation group

LAST_RESULTS = None


def build_kernel():
    nc = bacc.Bacc("TRN2", target_bir_lowering=False, debug=False, num_devices=NCORES)

    xT = nc.dram_tensor("xT", [E, S], BF16, kind="ExternalInput")
    wT = nc.dram_tensor("wT", [E, V3], BF16, kind="ExternalInput")
    bq = nc.dram_tensor("bq", [128, 4], F32, kind="ExternalInput")
    bv = nc.dram_tensor("bv", [1, 256], BF16, kind="ExternalInput")
    woT = nc.dram_tensor("woT", [2 * 128, E], BF16, kind="ExternalInput")
    out0 = nc.dram_tensor("out0", [S, E], F32, kind="ExternalOutput")
    out1 = nc.dram_tensor("out1", [S, E], F32, kind="ExternalOutput")

    with tile.TileContext(nc) as tc:
        with (
            tc.tile_pool(name="persist", bufs=1) as persist,
            tc.tile_pool(name="smalls", bufs=3) as smalls,
            tc.tile_pool(name="expp", bufs=2 * FG) as expp,
            tc.tile_pool(name="vsp", bufs=2 * FG + 2) as vsp,
            tc.tile_pool(name="fout", bufs=2) as foutp,
            tc.tile_pool(name="mm_ps", bufs=2, space="PSUM") as mm_ps,
            tc.tile_pool(name="sp_ps", bufs=2, space="PSUM") as sp_ps,
            tc.tile_pool(name="ot_ps", bufs=1, space="PSUM") as ot_ps,
        ):
            qk_sb = persist.tile([128, 4, S], BF16, tag="qk")
            v_sb = persist.tile([128, ST, 256], F32, tag="v")
            outT_f32 = persist.tile([128, 2, S], F32, tag="outT")
            outT_bf = persist.tile([128, 2, S], BF16, tag="outT_bf")
            bq_sb = persist.tile([128, 4], F32, tag="bq")
            bv_sb = persist.tile([1, 256], BF16, tag="bv")
            ones_sb = persist.tile([1, 512], BF16, tag="ones")
            xt_sb = persist.tile([128, ET, S], BF16, tag="xt")
            wt_sb = persist.tile([128, ET, V3], BF16, tag="wt")
            wo_sb = persist.tile([128, 2, E], BF16, tag="wo")

            nc.vector.memset(ones_sb[:], 1.0)
            for et in range(ET):
                nc.gpsimd.dma_start(wt_sb[:, et, :], wT[et * 128 : (et + 1) * 128, :])
            for sc in range(SC):
                for et in range(ET):
                    nc.sync.dma_start(
                        xt_sb[:, et, sc * 512 : (sc + 1) * 512],
                        xT[et * 128 : (et + 1) * 128, sc * 512 : (sc + 1) * 512],
                    )
            nc.gpsimd.dma_start(bq_sb[:], bq[:])
            nc.gpsimd.dma_start(bv_sb[:], bv[:])
            for p in range(2):
                nc.gpsimd.dma_start(wo_sb[:, p, :], woT[p * 128 : (p + 1) * 128, :])

            # ---- emitters for qkT / V accumulation groups ----------------
            def emit_qk_group(eo, sc):
                pt = mm_ps.tile([128, 512], F32, tag="mmps")
                for et in range(ET):
                    nc.tensor.matmul(
                        pt[:],
                        wt_sb[:, et, eo * 128 : (eo + 1) * 128],
                        xt_sb[:, et, sc * 512 : (sc + 1) * 512],
                        start=(et == 0),
                        stop=(et == ET - 1),
                    )
                nc.vector.tensor_scalar_add(
                    qk_sb[:, eo, sc * 512 : (sc + 1) * 512],
                    in0=pt[:],
                    scalar1=bq_sb[:, eo : eo + 1],
                )

            def emit_v_group(st):
                pt = mm_ps.tile([128, 512], F32, tag="mmps")
                for et in range(ET):
                    nc.tensor.matmul(
                        pt[:, :256],
                        xt_sb[:, et, st * 128 : (st + 1) * 128],
                        wt_sb[:, et, QK:V3],
                        start=(et == 0),
                        stop=False,
                    )
                nc.tensor.matmul(  # + ones^T bv (bias row)
                    pt[:, :256],
                    ones_sb[0:1, 0:128],
                    bv_sb[0:1, :],
                    start=False,
                    stop=True,
                )
                nc.vector.tensor_copy(v_sb[:, st, :], pt[:, :256])

            def emit_d_group(p, st, out_dram):
                ot = foutp.tile([128, E], F32, tag="fout", name=f"fo_{p}_{st}")
                for nck in range(2):
                    pt = mm_ps.tile([128, 512], F32, tag="mmps", name=f"fp_{p}_{st}_{nck}")
                    nc.tensor.matmul(
                        pt[:],
                        outT_bf[:, p, st * 128 : (st + 1) * 128],
                        wo_sb[:, p, nck * 512 : (nck + 1) * 512],
                        start=True,
                        stop=True,
                    )
                    if p == 1 and nck == 1:
                        nc.scalar.copy(ot[:, nck * 512 : (nck + 1) * 512], pt[:])
                    else:
                        nc.vector.tensor_copy(ot[:, nck * 512 : (nck + 1) * 512], pt[:])
                nc.sync.dma_start(out_dram[st * 128 : (st + 1) * 128, :], ot[:])

            # ---- pre-attention: just enough for pair0 kt0 ----------------
            # Emission order IS program order: every filler must be emitted
            # no later than the k-tile iteration that first consumes it
            # (fillers pop at the TOP of each k-tile iteration).
            emit_qk_group(0, 0)  # Q heads 0,1 cols 0-511
            emit_qk_group(0, 1)
            emit_qk_group(2, 0)  # K heads 0,1 cols 0-511 (kts 0-3)

            def qg(eo, sc):
                return lambda: emit_qk_group(eo, sc)

            def vg(st):
                return lambda: emit_v_group(st)

            fillers = (
                [vg(0), vg(1), qg(2, 1), vg(2), vg(3), qg(2, 2), vg(4), qg(2, 3)]
                + [vg(5), vg(6), vg(7), vg(8)]
                + [qg(1, 0), qg(1, 1), qg(1, 2), qg(1, 3)]
                + [vg(9), vg(10)]
                + [qg(3, 0), qg(3, 1)]
                + [vg(11), vg(12), vg(13), vg(14), vg(15)]
            )
            fillers.reverse()  # pop() from the front

            # ---- attention per head pair ---------------------------------
            # attn.V slices for group g are spread over group g+1's k-tiles
            # (2 of a half's 4 j-steps per k-tile) so the PE load per k-tile
            # is even and the exp stream never sees a burst.
            c_state = {}

            def emit_c_slices(p, g, half, jpair, exs, vss):
                if jpair == 0:
                    c_state[half] = ot_ps.tile(
                        [128, 1024], F32, tag="otps", name=f"oTt_{p}_{g}_{half}"
                    )
                oTt = c_state[half]
                for j in (2 * jpair, 2 * jpair + 1):
                    kt = FG * g + j
                    for hh in range(2):
                        for qc in range(2):
                            q0 = half * 1024 + qc * 512
                            nc.tensor.matmul(
                                oTt[
                                    hh * 64 : (hh + 1) * 64,
                                    qc * 512 : (qc + 1) * 512,
                                ],
                                vss[kt][:, hh, :],
                                exs[kt][:, hh, q0 : q0 + 512],
                                start=(j == 0),
                                stop=(j == FG - 1),
                            )
                if jpair == 1:
                    dst = outT_f32[:, p, half * 1024 : (half + 1) * 1024]
                    if g == 0:
                        nc.vector.tensor_copy(dst, oTt[:])
                    else:
                        nc.vector.tensor_add(dst, dst, oTt[:])

            for p in range(2):
                exs = {}
                vss = {}
                for kt in range(KT):
                    ex = expp.tile([128, 2, S], BF16, tag="exp")
                    exs[kt] = ex
                    den = smalls.tile([128, 2, 2], F32, tag="den")
                    for half in range(2):
                        for hh in range(2):
                            sp = sp_ps.tile([128, 1024], F32, tag="sp")
                            for qc in range(2):
                                q0 = half * 1024 + qc * 512
                                nc.tensor.matmul(
                                    sp[:, qc * 512 : (qc + 1) * 512],
                                    qk_sb[
                                        hh * 64 : (hh + 1) * 64,
                                        2 + p,
                                        kt * 128 : (kt + 1) * 128,
                                    ],
                                    qk_sb[hh * 64 : (hh + 1) * 64, p, q0 : q0 + 512],
                                    start=True,
                                    stop=True,
                                )
                            nc.scalar.activation(
                                ex[:, hh, half * 1024 : (half + 1) * 1024],
                                sp[:],
                                AF.Exp,
                                scale=0.125,
                                accum_out=den[:, hh, half : half + 1],
                            )
                        if p == 0 and kt == 0 and half == 0:
                            emit_qk_group(0, 2)  # Q cols 1024-2047 for half1
                            emit_qk_group(0, 3)
                    # previous group's attn.V, 8 matmuls per k-tile
                    if kt >= FG:
                        o = kt % FG
                        emit_c_slices(p, kt // FG - 1, o // 2, o % 2, exs, vss)
                    # PE fillers (producers before their consumers)
                    if p == 0:
                        for _ in range(2):
                            if fillers:
                                fillers.pop()()
                    elif kt < 2:  # pair1 kt0/1: remaining K tiles for heads 2,3
                        emit_qk_group(3, 2 + kt)
                    else:  # pair1: overlap pair0's projection
                        emit_d_group(0, kt - 2, out0)
                        if kt >= 14:
                            emit_d_group(0, kt - 2 + 2, out0)
                    dsum = smalls.tile([128, 2], F32, tag="dsum")
                    nc.vector.tensor_add(dsum[:], den[:, :, 0], den[:, :, 1])
                    rec = smalls.tile([128, 2], F32, tag="rec")
                    nc.vector.reciprocal(rec[:], dsum[:])
                    vs = vsp.tile([128, 2, DH], BF16, tag="vs")
                    vss[kt] = vs
                    for hh in range(2):
                        nc.vector.tensor_scalar_mul(
                            vs[:, hh, :],
                            in0=v_sb[:, kt, (2 * p + hh) * 64 : (2 * p + hh + 1) * 64],
                            scalar1=rec[:, hh : hh + 1],
                        )
                # tail: last group's attn.V (both q-halves), then the
                # projection; copies split across DVE and the idle ACT
                for half in range(2):
                    emit_c_slices(p, KT // FG - 1, half, 0, exs, vss)
                    emit_c_slices(p, KT // FG - 1, half, 1, exs, vss)
                    nc.vector.tensor_copy(
                        outT_bf[:, p, half * 1024 : (half + 1) * 1024],
                        outT_f32[:, p, half * 1024 : (half + 1) * 1024],
                    )
                if p == 1:
                    for st in range(ST):
                        emit_d_group(1, st, out1)


    nc.compile()
    return nc


def _shard_inputs(input, Wqkv, bqkv, Wo):
    """Build the 8 per-core input dicts (host-side layout/sharding)."""
    bf16 = ml_dtypes.bfloat16
    in_maps = []
    for c in range(NCORES):
        b = c // 4
        g = c % 4
        heads = range(4 * g, 4 * g + 4)
        rows = (
            [slice(64 * h, 64 * h + 64) for h in heads]
            + [slice(E + 64 * h, E + 64 * h + 64) for h in heads]
            + [slice(2 * E + 64 * h, 2 * E + 64 * h + 64) for h in heads]
        )
        W_sel = np.concatenate([Wqkv[s] for s in rows], axis=0)  # [768, 1024]
        b_sel = np.concatenate([bqkv[s] for s in rows], axis=0)  # [768]
        in_maps.append(
            {
                "xT": np.ascontiguousarray(input[b].T).astype(bf16),
                "wT": np.ascontiguousarray(W_sel.T).astype(bf16),
                "bq": np.ascontiguousarray(b_sel[:QK].reshape(4, 128).T),
                "bv": np.ascontiguousarray(b_sel[QK:V3].reshape(1, 256)).astype(bf16),
                "woT": np.ascontiguousarray(
                    Wo[:, 4 * g * DH : 4 * (g + 1) * DH].T
                ).astype(bf16),
            }
        )
    return in_maps


def kernel(input, Wqkv, bqkv, Wo, bo, _trace=False):
    global LAST_RESULTS
    input = np.asarray(input, dtype=np.float32)
    Wqkv = np.asarray(Wqkv, dtype=np.float32)
    bqkv = np.asarray(bqkv, dtype=np.float32)
    Wo = np.asarray(Wo, dtype=np.float32)
    bo = np.asarray(bo, dtype=np.float32)

    nc = build_kernel()
    in_maps = _shard_inputs(input, Wqkv, bqkv, Wo)
    kwargs = {}
    if _trace:
        kwargs = dict(trace=True, trace_cores=[0])
    res = run_bass_kernel_spmd(nc, in_maps, core_ids=list(range(NCORES)), **kwargs)
    LAST_RESULTS = res

    out = np.zeros((B, S, E), dtype=np.float32)
    for c in range(NCORES):
        out[c // 4] += res.results[c]["out0"]
        out[c // 4] += res.results[c]["out1"]
    out += bo
    return out

